# revision 15
# baseline (speedup 1.0000x reference)
"""KStepRGCN Trainium2 kernel: 8-core SPMD Bass/Tile implementation (v2).

Sharding: nodes partitioned into 8 dst-slices (graph-partition style).
Per layer, each core gathers the bf16 feature rows of its edges' sources
from a replicated HBM table (SWDGE dma_gather, pipelined with
consumer-side completion waits), segment-sums them into per-(tile,
relation) A^T blocks via PE one-hot matmuls whose one-hot S matrices are
generated on-chip (DVE iota==cv compare, batched per gather segment),
then applies the basis-decomposed relation transforms. The root term is
folded in as a 4th "relation": a PE transpose of the local h tile
against diag(cnt) so the subsequent 1/cnt mean scaling cancels. Between
layers the updated bf16 slices are AllGathered into the next table.
"""

import sys

sys.path.insert(0, "/opt/trn_rl_repo")

import numpy as np
import ml_dtypes

BF16 = ml_dtypes.bfloat16

# problem constants (hardcoded per harness contract)
N, E, D, R, B, K = 50000, 600000, 128, 3, 3, 3
NCORES = 8
LO_LIMIT = 32768
SEGC_LO, SEGC_HI = 32, 16  # chunks per gather segment, per index stream
NSELF = R  # slot index of the self/root pseudo-relation (blocks per tile = R+1)


class Cfg:
    def __init__(self, n=N, e=E, ncores=NCORES):
        assert n % ncores == 0
        self.n, self.e, self.ncores = n, e, ncores
        self.ns = n // ncores                 # real nodes per slice
        self.tpc = (self.ns + 127) // 128     # dst col tiles per core
        self.nsp = self.tpc * 128             # padded slice
        self.trows = ncores * self.nsp        # table rows
        self.nblk = R * self.tpc              # (tile, relation) blocks
        self.lo_lim = min(LO_LIMIT, self.trows)


def _pack_idx(rows_flat, nseg, segc):
    """rows_flat [nseg*segc*128] -> [128, nseg*segc*8] i16 (16p wrap, 8x rep)."""
    tot = nseg * segc
    arr = rows_flat.reshape(nseg, segc * 8, 16).transpose(0, 2, 1)
    arr = arr.reshape(nseg, 16, segc * 8).transpose(1, 0, 2).reshape(16, tot * 8)
    return np.tile(arr, (8, 1)).astype(np.int16)


def _preprocess(cfg, edge_index, edge_attr):
    """Shared (cross-core uniform) schedule + per-core gather/scatter data."""
    src = np.asarray(edge_index[0], dtype=np.int64)
    dst = np.asarray(edge_index[1], dtype=np.int64)
    attr = np.asarray(edge_attr, dtype=np.int64)
    ns, nsp, tpc, nblk, nc_ = cfg.ns, cfg.nsp, cfg.tpc, cfg.nblk, cfg.ncores
    lo_lim = cfg.lo_lim

    row = (src // ns) * nsp + (src % ns)
    reg = (row >= lo_lim).astype(np.int64)
    rowrel = row - reg * lo_lim
    core = dst // ns
    v = dst - core * ns
    bl = (v >> 7) * R + attr
    cvv = v & 127
    key = bl * 2 + reg

    counts = np.zeros((nc_, nblk, 2), dtype=np.int64)
    for c in range(nc_):
        counts[c] = np.bincount(key[core == c], minlength=nblk * 2).reshape(nblk, 2)
    cpb = -(-counts.max(axis=0) // 128)  # [nblk, 2] chunks per block (shared)

    lo_start = np.concatenate(([0], np.cumsum(cpb[:, 0])))
    hi_start = np.concatenate(([0], np.cumsum(cpb[:, 1])))
    CL, CH = int(lo_start[-1]), int(hi_start[-1])
    nlo_seg = max(1, -(-CL // SEGC_LO))
    nhi_seg = -(-CH // SEGC_HI) if CH else 0
    CLpad = nlo_seg * SEGC_LO
    CHpad = max(1, nhi_seg) * SEGC_HI

    per_core = []
    for c in range(nc_):
        m = core == c
        kb, rr, cvc = key[m], rowrel[m], cvv[m]
        o = np.argsort(kb, kind="stable")
        kb, rr, cvc = kb[o], rr[o], cvc[o]
        cnt_k = np.bincount(kb, minlength=nblk * 2)
        start_k = np.concatenate(([0], np.cumsum(cnt_k)))
        rank = np.arange(len(kb)) - start_k[kb]
        blv, regv = kb // 2, kb % 2
        stream_chunk = np.where(regv == 0, lo_start[blv], hi_start[blv]) + rank // 128
        slot = rank % 128
        rows_lo = np.zeros(CLpad * 128, dtype=np.int64)
        cv_lo = np.full(CLpad * 128, 255, dtype=np.int64)
        rows_hi = np.zeros(CHpad * 128, dtype=np.int64)
        cv_hi = np.full(CHpad * 128, 255, dtype=np.int64)
        is_lo = regv == 0
        pos_lo = stream_chunk[is_lo] * 128 + slot[is_lo]
        rows_lo[pos_lo] = rr[is_lo]
        cv_lo[pos_lo] = cvc[is_lo]
        pos_hi = stream_chunk[~is_lo] * 128 + slot[~is_lo]
        rows_hi[pos_hi] = rr[~is_lo]
        cv_hi[pos_hi] = cvc[~is_lo]
        per_core.append(dict(
            idx_lo=_pack_idx(rows_lo, nlo_seg, SEGC_LO)[:16],
            cv_lo=np.ascontiguousarray(cv_lo.reshape(CLpad, 128).T).astype(BF16),
            idx_hi=_pack_idx(rows_hi, max(1, nhi_seg), SEGC_HI)[:16],
            cv_hi=np.ascontiguousarray(cv_hi.reshape(CHpad, 128).T).astype(BF16),
        ))

    sched = dict(cpb=cpb, lo_start=lo_start, hi_start=hi_start,
                 nlo_seg=nlo_seg, nhi_seg=nhi_seg, lo_lim=lo_lim)
    return sched, per_core


def _build_program(cfg, sched, k_layers=K, prelu_a=0.25):
    from concourse import bacc, mybir
    import concourse.tile as tile
    from contextlib import ExitStack

    f32, bf16, i16 = mybir.dt.float32, mybir.dt.bfloat16, mybir.dt.int16
    Alu = mybir.AluOpType
    tpc, nsp, trows, lo_lim = cfg.tpc, cfg.nsp, cfg.trows, sched["lo_lim"]
    cpb, lo_start, hi_start = sched["cpb"], sched["lo_start"], sched["hi_start"]
    nlo_seg, nhi_seg = sched["nlo_seg"], sched["nhi_seg"]
    NR = R + 1  # relations + self

    nc = bacc.Bacc("TRN2", target_bir_lowering=False, debug=False,
                   num_devices=cfg.ncores, dynamic_dma_scratch_size=32768)

    x0_in = nc.dram_tensor("x0", [128, nsp], bf16, kind="ExternalInput")
    cnt_in = nc.dram_tensor("cnt_row", [1, nsp], bf16, kind="ExternalInput")
    cntc_in = nc.dram_tensor("cnt_col", [128, tpc], f32, kind="ExternalInput")
    w_in = nc.dram_tensor("w_sw", [128, k_layers * NR * D], bf16,
                          kind="ExternalInput")
    bias_in = nc.dram_tensor("bias_in", [1, k_layers * D], bf16,
                             kind="ExternalInput")
    invc_in = nc.dram_tensor("invc_in", [128, tpc], f32, kind="ExternalInput")
    iota_in = nc.dram_tensor("iota_in", [128, 128], bf16, kind="ExternalInput")
    pcol_in = nc.dram_tensor("pcol_in", [128, 1], f32, kind="ExternalInput")
    idx_lo_in = nc.dram_tensor("idx_lo", [16, nlo_seg * SEGC_LO * 8], i16,
                               kind="ExternalInput")
    cv_lo_in = nc.dram_tensor("cv_lo", [128, nlo_seg * SEGC_LO], bf16,
                              kind="ExternalInput")
    idx_hi_in = nc.dram_tensor("idx_hi", [16, max(1, nhi_seg) * SEGC_HI * 8],
                               i16, kind="ExternalInput")
    cv_hi_in = nc.dram_tensor("cv_hi", [128, max(1, nhi_seg) * SEGC_HI], bf16,
                              kind="ExternalInput")
    out_own = nc.dram_tensor("out_own", [nsp, D], bf16, kind="ExternalOutput")

    ag_in = nc.dram_tensor("ag_in", [nsp, D], bf16, kind="Internal")
    tables = [nc.dram_tensor(f"table{i}", [trows, D], bf16, kind="Internal",
                             addr_space="Shared") for i in range(k_layers)]
    rg = [list(range(cfg.ncores))]

    # per-tile chunk plan: (stream, seg, pos, r) in consumption order
    tiles_plan = []
    for t in range(tpc):
        chunks = []
        for r in range(R):
            b = t * R + r
            for j in range(int(cpb[b, 0])):
                cid = int(lo_start[b]) + j
                chunks.append(("lo", cid // SEGC_LO, cid % SEGC_LO, r))
            for j in range(int(cpb[b, 1])):
                cid = int(hi_start[b]) + j
                chunks.append(("hi", cid // SEGC_HI, cid % SEGC_HI, r))
        tiles_plan.append(chunks)

    from contextlib import ExitStack

    with tile.TileContext(nc) as tc, ExitStack() as ctx:
        const = ctx.enter_context(tc.tile_pool(name="const", bufs=1))
        hbf = const.tile([128, nsp], bf16, tag="hbf")
        cnt_t = const.tile([1, nsp], bf16, tag="cnt")
        cntc_t = const.tile([128, tpc], f32, tag="cntc")
        w_t = const.tile([128, k_layers * NR * D], bf16, tag="w")
        bias_t = const.tile([1, k_layers * D], bf16, tag="bias")
        invc_t = const.tile([128, tpc], f32, tag="invc")
        iota_t = const.tile([128, 128], bf16, tag="iota")
        pcol_t = const.tile([128, 1], f32, tag="pcol")
        ident_t = const.tile([128, 128], bf16, tag="ident")
        h_own = const.tile([128, nsp], bf16, tag="h_own")
        idx_lo_t = const.tile([128, nlo_seg * SEGC_LO * 8], i16, tag="ixl")
        cv_lo_t = const.tile([128, nlo_seg * SEGC_LO], bf16, tag="cvl")
        idx_hi_t = const.tile([128, max(1, nhi_seg) * SEGC_HI * 8], i16,
                              tag="ixh")
        cv_hi_t = const.tile([128, max(1, nhi_seg) * SEGC_HI], bf16, tag="cvh")

        nc.sync.dma_start(hbf[:], x0_in.ap())
        nc.sync.dma_start(cnt_t[:], cnt_in.ap())
        nc.sync.dma_start(cntc_t[:], cntc_in.ap())
        nc.sync.dma_start(w_t[:], w_in.ap())
        nc.sync.dma_start(bias_t[:], bias_in.ap())
        nc.sync.dma_start(invc_t[:], invc_in.ap())
        nc.sync.dma_start(iota_t[:], iota_in.ap())
        nc.sync.dma_start(pcol_t[:], pcol_in.ap())
        for gidx8 in range(8):
            nc.sync.dma_start(idx_lo_t[gidx8 * 16:(gidx8 + 1) * 16, :],
                              idx_lo_in.ap())
            if nhi_seg:
                nc.sync.dma_start(idx_hi_t[gidx8 * 16:(gidx8 + 1) * 16, :],
                                  idx_hi_in.ap())
        nc.sync.dma_start(cv_lo_t[:], cv_lo_in.ap())
        if nhi_seg:
            nc.sync.dma_start(cv_hi_t[:], cv_hi_in.ap())
        # identity matrix: ident[p, f] = (f == p)
        nc.vector.tensor_scalar(ident_t[:], iota_t[:], pcol_t[:, 0:1], None,
                                Alu.is_equal)

        msg_lo = ctx.enter_context(tc.tile_pool(name="msg_lo", bufs=3))
        msg_hi = ctx.enter_context(tc.tile_pool(name="msg_hi", bufs=3))
        sp_lo = ctx.enter_context(tc.tile_pool(name="sp_lo", bufs=3))
        sp_hi = ctx.enter_context(tc.tile_pool(name="sp_hi", bufs=3))
        self_p = ctx.enter_context(tc.tile_pool(name="selfp", bufs=2))
        a_pool = ctx.enter_context(tc.tile_pool(name="aT", bufs=3))
        tmp_pool = ctx.enter_context(tc.tile_pool(name="tmp", bufs=2))
        pblk = ctx.enter_context(tc.tile_pool(name="pblk", bufs=2, space="PSUM"))
        pout = ctx.enter_context(tc.tile_pool(name="pout", bufs=2, space="PSUM"))

        prep_sem = ctx.enter_context(nc.semaphore())
        dma_sem = ctx.enter_context(nc.semaphore())
        gcount = [0]

        segcfg = dict(
            lo=(msg_lo, sp_lo, idx_lo_t, cv_lo_t, SEGC_LO, 0, lo_lim),
            hi=(msg_hi, sp_hi, idx_hi_t, cv_hi_t, SEGC_HI, lo_lim, trows))

        for k in range(k_layers):
            # ---- AllGather h -> table[k] (ag_in written at k=0 here, else
            # streamed out per 4-tile group during layer k-1)
            if k == 0:
                nc.sync.dma_start(
                    ag_in.ap().rearrange("(t p) f -> p t f", p=128),
                    hbf[:].rearrange("p (t f) -> p t f", f=D))
            nc.gpsimd.collective_compute(
                "AllGather", Alu.bypass, replica_groups=rg,
                ins=[ag_in.ap()], outs=[tables[k].ap()])
            table = tables[k]

            seg_tiles = {}   # (stream, seg) -> (mt, st, g)

            def emit_seg(stream, s, k=k, table=table, seg_tiles=seg_tiles):
                if (stream, s) in seg_tiles:
                    return
                mpool, spool, idx_t, cv_t, segc, r0, r1 = segcfg[stream]
                mt = mpool.tile([128, segc, D], bf16, tag="m")
                gcount[0] += 1
                g = gcount[0]
                with tc.tile_critical():
                    nc.gpsimd.dma_gather(
                        out_ap=mt[:], in_ap=table.ap()[r0:r1, :],
                        idxs_ap=idx_t[:, s * segc * 8:(s + 1) * segc * 8],
                        num_idxs=segc * 128, num_idxs_reg=segc * 128,
                        elem_size=D, prepare_only=True, sem=dma_sem,
                        single_packet=False).then_inc(prep_sem, 1)
                    nc.gpsimd.wait_ge(prep_sem, g)
                    nc.gpsimd.trigger_dma(count=1)
                st = spool.tile([128, segc * 128], bf16, tag="s")
                st_v = st[:].rearrange("p (c f) -> p c f", f=128)
                io_b = iota_t[:].rearrange("p (o f) -> p o f", o=1).broadcast_to(
                    [128, segc, 128])
                cv_b = cv_t[:, s * segc:(s + 1) * segc].rearrange(
                    "p (c o) -> p c o", o=1).broadcast_to([128, segc, 128])
                nc.vector.tensor_tensor(st_v, io_b, cv_b, Alu.is_equal)
                seg_tiles[(stream, s)] = (mt, st, g)

            for t in range(tpc):
                chunks = tiles_plan[t]
                for stream, s, pos, r in chunks:
                    emit_seg(stream, s)
                tsl = slice(t * 128, (t + 1) * 128)
                pb = pblk.tile([128, NR * 128], f32, tag="pb")
                nprev = {}
                ntot = {r: sum(1 for c in chunks if c[3] == r) for r in range(R)}
                gmax = max((seg_tiles[(st_, s_)][2] for st_, s_, _, _ in chunks),
                           default=0)
                # self/root block operand: local h tile pre-scaled by cnt
                tself = self_p.tile([128, 128], bf16, tag="ts")
                nc.vector.tensor_scalar(tself[:], hbf[:, tsl],
                                        cntc_t[:, t:t + 1], None, Alu.mult)
                with tc.tile_critical():
                    if chunks:
                        nc.tensor.wait_ge(dma_sem, 16 * gmax)
                    for stream, s, pos, r in chunks:
                        mt, st, _ = seg_tiles[(stream, s)]
                        done = nprev.get(r, 0)
                        nc.tensor.matmul(
                            pb[:, r * 128:(r + 1) * 128], lhsT=mt[:, pos, :],
                            rhs=st[:, pos * 128:(pos + 1) * 128],
                            start=(done == 0), stop=(done == ntot[r] - 1))
                        nprev[r] = done + 1
                    # transpose cnt-scaled h tile into the self block
                    nc.tensor.matmul(pb[:, NSELF * 128:(NSELF + 1) * 128],
                                     lhsT=tself[:], rhs=ident_t[:],
                                     start=True, stop=True)
                aT = a_pool.tile([128, NR * 128], bf16, tag="a")
                nc.scalar.copy(aT[:], pb[:])

                # ---- transform: po[n,f'] = sum_r A_r W_r + cnt*(h root + bias)
                gpos = t % 4
                if gpos == 0:
                    pout_g = pout.tile([128, 512], f32, tag="po")
                    g0 = t
                po = pout_g[:, gpos * 128:(gpos + 1) * 128]
                first = True
                for r in range(R):
                    if ntot[r]:
                        nc.tensor.matmul(
                            po, lhsT=aT[:, r * 128:(r + 1) * 128],
                            rhs=w_t[:, (k * NR + r) * D:(k * NR + r + 1) * D],
                            start=first, stop=False)
                        first = False
                nc.tensor.matmul(
                    po, lhsT=aT[:, NSELF * 128:(NSELF + 1) * 128],
                    rhs=w_t[:, (k * NR + NSELF) * D:(k * NR + NSELF + 1) * D],
                    start=first, stop=False)
                nc.tensor.matmul(po, lhsT=cnt_t[:, tsl],
                                 rhs=bias_t[:, k * D:(k + 1) * D],
                                 start=False, stop=True)

                if gpos == 3 or t == tpc - 1:
                    ngr = gpos + 1
                    used = ngr * 128
                    po_v = pout_g[:, :used].rearrange("p (c f) -> p c f", f=128)
                    invc_b = invc_t[:, g0:g0 + ngr].rearrange(
                        "p (c o) -> p c o", o=1).broadcast_to([128, ngr, 128])
                    base = g0 * 128
                    if k < k_layers - 1:
                        tmp = tmp_pool.tile([128, 512], f32, tag="t")
                        tmp_v = tmp[:, :used].rearrange("p (c f) -> p c f", f=128)
                        nc.vector.tensor_tensor(tmp_v, po_v, invc_b, Alu.mult)
                        nc.vector.scalar_tensor_tensor(
                            hbf[:, base:base + used], tmp[:, :used], prelu_a,
                            tmp[:, :used], Alu.mult, Alu.max)
                        nc.sync.dma_start(
                            ag_in.ap()[g0 * 128:(g0 + ngr) * 128, :].rearrange(
                                "(t p) f -> p t f", p=128),
                            hbf[:, base:base + used].rearrange(
                                "p (t f) -> p t f", f=D))
                    else:
                        ho_v = h_own[:, base:base + used].rearrange(
                            "p (c f) -> p c f", f=128)
                        nc.vector.tensor_tensor(ho_v, po_v, invc_b, Alu.mult)

        nc.sync.dma_start(out_own.ap().rearrange("(t p) f -> p t f", p=128),
                          h_own[:].rearrange("p (t f) -> p t f", f=D))

    nc.compile()
    return nc


def _host_tensors(cfg, sched, per_core, edge_index, x, basis, att, root, bias,
                  k_layers=K):
    ns, nsp, tpc = cfg.ns, cfg.nsp, cfg.tpc
    NR = R + 1
    dst = np.asarray(edge_index[1], dtype=np.int64)
    cntp = np.maximum(np.bincount(dst, minlength=cfg.n), 1).astype(np.float32)

    W = np.einsum("krb,kbio->krio", np.asarray(att, np.float32),
                  np.asarray(basis, np.float32))[:k_layers]        # [k,R,D,D]
    Wfull = np.concatenate(
        [W, np.asarray(root, np.float32)[:k_layers, None]], axis=1)  # [k,NR,D,D]
    w_sw = np.ascontiguousarray(
        Wfull.transpose(2, 0, 1, 3).reshape(D, k_layers * NR * D)).astype(BF16)
    bias_in = np.asarray(bias, np.float32)[:k_layers].reshape(
        1, k_layers * D).astype(BF16)
    iota = np.tile(np.arange(128, dtype=np.float32), (128, 1)).astype(BF16)

    x = np.asarray(x, dtype=np.float32)
    pcol = np.arange(128, dtype=np.float32).reshape(128, 1)
    in_maps = []
    for c in range(cfg.ncores):
        xp = np.zeros((nsp, D), dtype=np.float32)
        xp[:ns] = x[c * ns:(c + 1) * ns]
        x0 = np.ascontiguousarray(
            xp.reshape(tpc, 128, D).transpose(1, 0, 2).reshape(128, nsp)
        ).astype(BF16)
        cnt_sl = np.zeros(nsp, dtype=np.float32)
        cnt_sl[:ns] = cntp[c * ns:(c + 1) * ns]
        inv_sl = np.ones(nsp, dtype=np.float32)
        inv_sl[:ns] = 1.0 / cntp[c * ns:(c + 1) * ns]
        pc = per_core[c]
        in_maps.append(dict(
            x0=x0,
            cnt_row=cnt_sl.reshape(1, nsp).astype(BF16),
            cnt_col=np.ascontiguousarray(cnt_sl.reshape(tpc, 128).T),
            w_sw=w_sw, bias_in=bias_in,
            invc_in=np.ascontiguousarray(inv_sl.reshape(tpc, 128).T),
            iota_in=iota, pcol_in=pcol,
            idx_lo=pc["idx_lo"], cv_lo=pc["cv_lo"],
            idx_hi=pc["idx_hi"], cv_hi=pc["cv_hi"]))
    return in_maps


def _run(cfg, x, edge_index, edge_attr, basis, att, root, bias, prelu_a,
         k_layers=K, trace=False):
    from concourse.bass_utils import run_bass_kernel_spmd

    sched, per_core = _preprocess(cfg, edge_index, edge_attr)
    nc = _build_program(cfg, sched, k_layers,
                        float(np.asarray(prelu_a).ravel()[0]))
    in_maps = _host_tensors(cfg, sched, per_core, edge_index, x, basis, att,
                            root, bias, k_layers)
    res = run_bass_kernel_spmd(nc, in_maps, core_ids=list(range(cfg.ncores)),
                               trace=trace)
    out = np.empty((cfg.n, D), dtype=np.float32)
    for c in range(cfg.ncores):
        out[c * cfg.ns:(c + 1) * cfg.ns] = (
            res.results[c]["out_own"][:cfg.ns].astype(np.float32))
    return out, res


def kernel(x, edge_index, edge_attr, basis, att, root, bias, prelu_a):
    cfg = Cfg()
    out, _ = _run(cfg, x, edge_index, edge_attr, basis, att, root, bias, prelu_a)
    return out
